# revision 13
# baseline (speedup 1.0000x reference)
"""Expert-parallel MoE MLP (ExpertMLP) Bass kernel for 8 Trainium2 NeuronCores.

Problem: x[32,4096,256] @ w_fc[32,256,1024] -> gelu(erf) -> @ w_proj[32,1024,256].

Sharding: expert-parallel. Each of the 8 cores gets 4 experts (slices of the
leading axis of every tensor); no cross-core communication. Inside a core, per
expert e:

  1. xT [d, c] in bf16 is produced with NO casting pass: the f32 input is
     bitcast to a bf16 view [cap, 2d] (u16 column pairs = lo/hi halves of each
     f32 word), 128-column blocks of that view are XBar DMA-transposed into
     SBUF tiles whose partitions interleave lo/hi, and a partition-strided
     SBUF->SBUF DMA (tu[1::2]) packs the hi halves - exactly the bf16
     truncations - into xT tiles. This removes the DRAM->DRAM gpsimd cast
     stage whose queue drain serialized ~60us ahead of the first matmul.
  2. MM1: hT[h_tile, c_chunk] += w_fc_tile.T @ xT_chunk - w_fc's natural
     [d, h] layout is the stationary operand (DVE-cast to bf16 on load).
  3. GELU (exact erf form) runs on the ACT engine as the PSUM->SBUF eviction.
  4. MM2 runs k-major over all capacity subtiles of the chunk: for k, for s:
     out[s] += hT[k][s].T @ w_proj[k]. The first MM2 instructions only need
     the chunk's first GELU group, so the PE never waits for the last GELU.
     Each subtile accumulator owns a FULL 2KB PSUM bank (start=True clears
     the whole bank). Results land directly in [capacity, d] orientation.

Transposes for expert e+1 are issued interleaved with expert e's chunk loop
(2 transposes + 2 deinterleaves per chunk) so the SP queue serves output DMAs
promptly and xT production stays ~1 expert ahead of consumption.

All matmul operands are bf16; PSUM accumulation stays fp32.
"""

import numpy as np
from contextlib import ExitStack

import bass_rust as _br
import concourse.bass as bass
import concourse.tile as tile
from concourse import mybir
from concourse.bass_utils import run_bass_kernel_spmd

E, CAP, D, H = 32, 4096, 256, 1024
N_CORES = 8
E_PER = E // N_CORES  # 4 experts per core
P = 128
F32 = mybir.dt.float32
BF16 = mybir.dt.bfloat16

KD = D // P        # 2 k-tiles in MM1's contraction
KH = H // P        # 8 k-tiles in MM2's contraction
NC_CHUNK = 512     # capacity chunk processed per MM1/MM2 round
N_CHUNKS = CAP // NC_CHUNK
H_TILES = H // P
SLAB = 1024        # DMA-transpose slab (capacity columns)
N_SLABS = CAP // SLAB
NB = 2 * KD        # 4 u16-column blocks of 128 per slab


def _fix_waits(nc):
    """walrus accepts only one sync wait per instruction (and none at all on
    gpsimd DIRECT2D DMAs); hoist excess waits onto standalone EventSemaphore
    instructions inserted before the offender (same engine => same sequencer
    order)."""
    for fn in nc.m.functions:
        for bb in fn.blocks:
            new = []
            changed = False
            for inst in bb.instructions:
                si = inst.sync_info
                keep = 0 if (
                    si is not None
                    and inst.engine == mybir.EngineType.Pool
                    and type(inst).__name__ in ("InstDMACopy", "InstTensorCopy")
                    and len(si.on_wait) >= 1
                ) else 1
                if si is not None and len(si.on_wait) > keep:
                    waits = list(si.on_wait)
                    for w in waits[: len(waits) - keep]:
                        ev = mybir.InstEventSemaphore(
                            name=nc.get_next_instruction_name()
                        )
                        ev.engine = inst.engine
                        ev.sync_info = _br.SyncInfo(on_wait=[w], on_update=[])
                        nc.register_instruction(ev)
                        new.append(ev)
                    inst.sync_info = _br.SyncInfo(
                        on_wait=waits[len(waits) - keep:],
                        on_update=list(si.on_update),
                    )
                    changed = True
                new.append(inst)
            if changed:
                bb.instructions = new


def _build():
    nc = bass.Bass(trn_type="TRN2", target_bir_lowering=False, debug=False)
    x = nc.dram_tensor("x", [E_PER, CAP, D], F32, kind="ExternalInput").ap()
    w_fc = nc.dram_tensor("w_fc", [E_PER, D, H], F32, kind="ExternalInput").ap()
    w_proj = nc.dram_tensor("w_proj", [E_PER, H, D], F32, kind="ExternalInput").ap()
    out = nc.dram_tensor("out", [E_PER, CAP, D], F32, kind="ExternalOutput").ap()
    xu = [x[e].bitcast(BF16) for e in range(E_PER)]  # [CAP, 2*D] u16-pair view

    with tile.TileContext(nc) as tc, ExitStack() as ctx:
        tup = ctx.enter_context(tc.tile_pool(name="tup", bufs=8))
        xtp = ctx.enter_context(tc.tile_pool(name="xtp", bufs=2 * N_SLABS * KD))
        wload = ctx.enter_context(tc.tile_pool(name="wload", bufs=2))
        wfc_p = ctx.enter_context(tc.tile_pool(name="wfc", bufs=2))
        wproj_p = ctx.enter_context(tc.tile_pool(name="wproj", bufs=2))
        ht_p = ctx.enter_context(tc.tile_pool(name="ht", bufs=8))
        out_p = ctx.enter_context(tc.tile_pool(name="outp", bufs=3))
        ps_h = ctx.enter_context(tc.tile_pool(name="ps_h", bufs=2, space="PSUM"))
        ps_o = ctx.enter_context(tc.tile_pool(name="ps_o", bufs=4, space="PSUM"))

        HPACK = 2          # h_tiles packed per PSUM tile / GELU call

        def load_weights(e):
            wfc_raw = wload.tile([P, KD, H], F32, tag="wl")
            nc.sync.dma_start(wfc_raw[:], w_fc[e].rearrange("(k p) h -> p k h", p=P))
            wfc = wfc_p.tile([P, KD, H], BF16, tag="wfc")
            nc.vector.tensor_copy(wfc[:], wfc_raw[:])
            wproj_raw = wload.tile([P, KH, D], F32, tag="wl")
            nc.sync.dma_start(
                wproj_raw[:], w_proj[e].rearrange("(k p) d -> p k d", p=P)
            )
            wproj = wproj_p.tile([P, KH, D], BF16, tag="wproj")
            nc.vector.tensor_copy(wproj[:], wproj_raw[:])
            return wfc, wproj

        def make_xt(e):
            return [
                [
                    xtp.tile([P, SLAB], BF16, tag="xt", name=f"xt{e}_{k}_{s}")
                    for s in range(N_SLABS)
                ]
                for k in range(KD)
            ]

        tus = {}

        def issue_transposes(e, s, half, split=False):
            """Transpose u16-block pair (2*half, 2*half+1) of slab s."""
            cs = slice(s * SLAB, (s + 1) * SLAB)
            for j in range(2):
                b = 2 * half + j
                tu = tup.tile([P, SLAB], BF16, tag="tu", name=f"tu{e}_{s}_{b}")
                eng = nc.scalar if (split and j == 1) else nc.sync
                eng.dma_start_transpose(tu[:], xu[e][cs, P * b:P * (b + 1)])
                tus[(e, s, b)] = tu

        def issue_deints(e, xt, s, half):
            """Pack the hi (odd) partitions of the block pair into the
            k=half xT tile's partition halves."""
            for j in range(2):
                tu = tus.pop((e, s, 2 * half + j))
                nc.scalar.dma_start(
                    xt[half][s][64 * j:64 * (j + 1), :], tu[1::2, :]
                )

        # prologue: expert 0's weights + xT, first slab split across both
        # HWDGE queues for the shortest path to the first matmul.
        w = [None] * E_PER
        xts = [None] * E_PER
        w[0] = load_weights(0)
        xts[0] = make_xt(0)
        for s in range(N_SLABS):
            for half in range(KD):
                issue_transposes(0, s, half, split=(s == 0))
                issue_deints(0, xts[0], s, half)

        for e in range(E_PER):
            xt = xts[e]
            wfc, wproj = w[e]

            for nci in range(N_CHUNKS):
                # stage expert e+1's xT one half-slab per chunk (2 transposes
                # + 2 deints, the deints one chunk behind their transposes so
                # their waits are satisfied and never block GELUs on the ACT
                # queue), and its weights at the first chunk.
                if e + 1 < E_PER:
                    if nci == 0:
                        w[e + 1] = load_weights(e + 1)
                        xts[e + 1] = make_xt(e + 1)
                    issue_transposes(e + 1, nci // KD, nci % KD)
                    if nci > 0:
                        issue_deints(e + 1, xts[e + 1], (nci - 1) // KD, (nci - 1) % KD)

                csl = slice(nci * NC_CHUNK, (nci + 1) * NC_CHUNK)
                sidx = (nci * NC_CHUNK) // SLAB
                soff = (nci * NC_CHUNK) % SLAB
                # ---- MM1 + GELU: HPACK h_tiles per PSUM tile / GELU call ----
                ht_tiles = []
                for hp in range(H_TILES // HPACK):
                    psh = ps_h.tile([P, HPACK, NC_CHUNK], F32, tag="psh")
                    for j in range(HPACK):
                        hi = hp * HPACK + j
                        for k in range(KD):
                            nc.tensor.matmul(
                                psh[:, j, :],
                                wfc[:, k, hi * P:(hi + 1) * P],
                                xt[k][sidx][:, soff:soff + NC_CHUNK],
                                start=(k == 0),
                                stop=(k == KD - 1),
                            )
                    ht = ht_p.tile([P, HPACK, NC_CHUNK], BF16, tag="ht")
                    nc.scalar.activation(
                        ht[:], psh[:], mybir.ActivationFunctionType.Gelu
                    )
                    ht_tiles.append(ht)

                # ---- MM2, k-major: psum[s] += hT[k][s].T @ w_proj[k] ----
                NS = NC_CHUNK // P  # 4 capacity subtiles
                psos = [
                    ps_o.tile([P, 2 * D], F32, tag="pso", name=f"pso{e}_{nci}_{i}")
                    for i in range(NS)
                ]
                for k in range(KH):
                    for s in range(NS):
                        nc.tensor.matmul(
                            psos[s][:, :D],
                            ht_tiles[k // HPACK][:, k % HPACK, s * P:(s + 1) * P],
                            wproj[:, k, :],
                            start=(k == 0),
                            stop=(k == KH - 1),
                        )
                ob = out_p.tile([P, NC_CHUNK // P, D], F32, tag="ob")
                for s, pso in enumerate(psos):
                    nc.vector.tensor_copy(ob[:, s, :], pso[:, :D])
                nc.sync.dma_start(
                    out[e, csl, :].rearrange("(s p) d -> p s d", p=P), ob[:]
                )

            if e + 1 < E_PER:
                issue_deints(e + 1, xts[e + 1], N_SLABS - 1, KD - 1)

    _fix_waits(nc)
    return nc


_CACHE = {}


def _get_nc():
    if "nc" not in _CACHE:
        _CACHE["nc"] = _build()
    return _CACHE["nc"]


def kernel(x, w_fc, w_proj, trace=False):
    assert x.shape == (E, CAP, D) and w_fc.shape == (E, D, H)
    assert w_proj.shape == (E, H, D)
    nc = _get_nc()
    x = np.ascontiguousarray(x, dtype=np.float32)
    w_fc = np.ascontiguousarray(w_fc, dtype=np.float32)
    w_proj = np.ascontiguousarray(w_proj, dtype=np.float32)
    in_maps = [
        {
            "x": x[i * E_PER:(i + 1) * E_PER],
            "w_fc": w_fc[i * E_PER:(i + 1) * E_PER],
            "w_proj": w_proj[i * E_PER:(i + 1) * E_PER],
        }
        for i in range(N_CORES)
    ]
    res = run_bass_kernel_spmd(nc, in_maps, list(range(N_CORES)), trace=trace)
    out = np.concatenate([r["out"] for r in res.results], axis=0)
    if trace:
        kernel.last_results = res
    return out
